# revision 18
# baseline (speedup 1.0000x reference)
"""Multi-head attention (B=2, S=2048, D=1024, H=16) on 8 Trainium2 cores.

Sharding: batch x head-group. Core c handles batch b = c//4 and heads
4*(c%4) .. 4*(c%4)+4 (a 256-wide slice of the feature dim). Each core:
  - projects q/k/v for its batch with its weight slices (transposed
    projections qhT/khT [d, s]; vh natural [s, d]),
  - computes scores transposed [k, q] on PE, softmax exp on ScalarE (no
    max subtraction needed: |scaled scores| <= ~0.6 by construction),
  - denominator via a ones-column appended to vh (row 64 of the attnV
    PSUM output); reciprocal runs on an [8,128] DRAM-bounced layout
    (DVE divide is per-lane-serial, so the [1,512] row shape is slow);
    the reciprocal row is partition-broadcast with a ones-stationary
    PE matmul instead of a DMA broadcast (saves 2MB of HBM reads),
  - attn @ V accumulated over k-tiles in PSUM, normalized at evacuation,
  - output projection into a partial y [2048, 1024] bf16, summed on host.

The kernel is HBM-bandwidth-bound (~14MB reads + 4MB writes per core per
run against a shared ~<1TB/s device). DMA policy: few and large
transfers (whole-tensor loads into resident SBUF tiles), x streams split
across the two HWDGE rings (xq+weights on SP, xk+xv on ACT).

Engine notes (measured): matmuls all bf16 (~44ns per [128x128x512] vs
126ns f32r); ScalarE runs only the 128 exps ([128,1024] f32->bf16,
~0.75us each); projection bias evacuation on DVE (tensor_scalar_add).
"""

import copy
import sys

import numpy as np

if "/opt/trn_rl_repo" not in sys.path:
    sys.path.insert(0, "/opt/trn_rl_repo")

B = 2
S = 2048
DIM = 1024
NH = 16
HD = 64
NCORES = 8
GROUPS = NCORES // B          # 4 head-groups per batch
HPC = NH // GROUPS            # 4 heads per core
CS = HPC * HD                 # 256-wide feature slice per core
PAIRS = HPC // 2              # head pairs per core

_PROGRAM = None


def _split_waits(nc, templates, max_waits=1):
    """This walrus build rejects instructions carrying more than one sync-wait
    command (verified for MATMUL/LDW, ACTIVATE, DMACopy and CTRL structs
    alike). Move excess waits onto injected same-engine NOPs placed right
    before the over-subscribed instruction (conditions are checked in the same
    engine-stream position, so semantics are unchanged)."""
    from concourse import mybir

    n_nops = 0
    for f in nc.m.functions:
        for blk in f.blocks:
            insts = blk.instructions
            i = 0
            while i < len(insts):
                inst = insts[i]
                si = inst.sync_info
                if si is not None and si.on_wait and len(si.on_wait) > max_waits:
                    waits = list(si.on_wait)
                    keep = waits[-max_waits:]
                    extra = waits[:-max_waits]
                    nops = []
                    for w in extra:
                        nop = copy.deepcopy(templates[inst.engine])
                        nop.name = f"waitnop-{n_nops}"
                        nop.sync_info = mybir.SyncInfo(on_wait=[w], on_update=[])
                        nops.append(nop)
                        n_nops += 1
                    inst.sync_info = mybir.SyncInfo(
                        on_wait=keep, on_update=list(si.on_update))
                    insts[i:i] = nops
                    i += len(nops)
                i += 1
    return n_nops


def _build_program(split=True, loop_n=1,
                   phases=("qk", "qkmm", "qkev", "v", "att", "evac", "fin")):
    import concourse.bass as bass
    import concourse.tile as tile
    from concourse import mybir

    dt = mybir.dt
    f32 = dt.float32
    bf16 = dt.bfloat16
    f16 = dt.float16
    ACT = mybir.ActivationFunctionType

    nc = bass.Bass()
    nop_templates = {
        eng.engine: eng.nop().ins
        for eng in (nc.tensor, nc.vector, nc.scalar, nc.gpsimd, nc.sync)
    }

    f8 = dt.float8e4
    xqT = nc.dram_tensor("xqT", [DIM, S], f8, kind="ExternalInput")
    xkT = nc.dram_tensor("xkT", [DIM, S], f8, kind="ExternalInput")
    xvT = nc.dram_tensor("xvT", [DIM, S], bf16, kind="ExternalInput")
    wqT = nc.dram_tensor("wqT", [DIM, CS], f8, kind="ExternalInput")
    wkT = nc.dram_tensor("wkT", [DIM, CS], f8, kind="ExternalInput")
    wvT = nc.dram_tensor("wvT", [DIM, CS], bf16, kind="ExternalInput")
    woT = nc.dram_tensor("woT", [CS, DIM], bf16, kind="ExternalInput")
    bq_s = nc.dram_tensor("bq_s", [CS, 1], f32, kind="ExternalInput")
    bk_s = nc.dram_tensor("bk_s", [CS, 1], f32, kind="ExternalInput")
    bv_s = nc.dram_tensor("bv_s", [1, CS], f32, kind="ExternalInput")
    ones_c = nc.dram_tensor("ones_c", [1, (S // 128) * HPC], bf16,
                            kind="ExternalInput")
    y = nc.dram_tensor("y", [S, DIM], bf16, kind="ExternalOutput")

    KT_PROJ = DIM // 128      # 8 contraction tiles for projections
    KT_ATT = S // 128         # 16 k-position tiles for attention
    ST = S // 128             # 16 s-tiles
    SCALE = 1.0 / np.sqrt(np.float32(DIM))
    # Q/K path is fp8 with x and W each pre-scaled by 32 on the host (to
    # land in e4m3's normal range); scores come out 1024x too big and the
    # exp free-affine scale absorbs it exactly (powers of two).
    SCALE_F8 = SCALE / (1024.0 * 1024.0)

    import contextlib

    with tile.TileContext(nc) as tc:
        loop_cm = (
            tc.For_i(0, loop_n, 1, hint_engines=(
                mybir.EngineType.PE, mybir.EngineType.Activation,
                mybir.EngineType.DVE, mybir.EngineType.SP))
            if loop_n > 1 else contextlib.nullcontext())
        with (
            loop_cm,
            tc.tile_pool(name="weights", bufs=1) as wpool,
            tc.tile_pool(name="persist", bufs=1) as persist,
            tc.tile_pool(name="exp", bufs=6) as expool,
            tc.tile_pool(name="small", bufs=2) as spool,
            tc.tile_pool(name="yout", bufs=2) as ypool,
        ):
            # ---- SBUF allocations ----
            wq_sb = wpool.tile([128, KT_PROJ, CS], f8, tag="wq")
            wk_sb = wpool.tile([128, KT_PROJ, CS], f8, tag="wk")
            wv_sb = wpool.tile([128, KT_PROJ, CS], bf16, tag="wv")
            wo_sb = wpool.tile([128, PAIRS, DIM], bf16, tag="wo")
            bq_sb = wpool.tile([128, PAIRS], f32, tag="bq")
            bk_sb = wpool.tile([128, PAIRS], f32, tag="bk")
            vb_sb = wpool.tile([128, CS], f32, tag="vb")
            ones_h = wpool.tile([1, 64], f16, tag="ones_h")
            # x inputs fully resident (32KB/partition each)
            xq_sb = wpool.tile([128, KT_PROJ, S], f8, tag="xq")
            xk_sb = wpool.tile([128, KT_PROJ, S], f8, tag="xk")
            xv_sb = wpool.tile([128, KT_PROJ, S], bf16, tag="xvs")
            qhT = persist.tile([128, PAIRS, S], bf16, tag="qhT")
            khT = persist.tile([128, PAIRS, S], bf16, tag="khT")
            # vh: per s-tile, per head: 64 cols of v plus a ones column (the
            # softmax denominator emerges as row 64 of the attnV output).
            vh = persist.tile([128, ST, HPC, HD + 1], bf16, tag="vh")
            out_sT = persist.tile([128, PAIRS, S], bf16, tag="out_sT")

            nc.vector.memset(ones_h[:], 1.0)

            # ---- input loads: few, large DMAs. SP ring: weights + xq (+ the
            # small constants); ACT ring: xk then xv. Each x tensor lands in
            # two [128, 4, 2048] halves so projection matmuls can start on
            # the first half.
            for pr in range(PAIRS):
                nc.sync.dma_start(bq_sb[:, pr:pr + 1],
                                  bq_s[pr * 128:(pr + 1) * 128, :])
                nc.sync.dma_start(bk_sb[:, pr:pr + 1],
                                  bk_s[pr * 128:(pr + 1) * 128, :])
            xq_r = xqT.rearrange("(t p) s -> p t s", p=128)
            xk_r = xkT.rearrange("(t p) s -> p t s", p=128)
            xv_r = xvT.rearrange("(t p) s -> p t s", p=128)
            wq_r = wqT.rearrange("(t p) c -> p t c", p=128)
            wk_r = wkT.rearrange("(t p) c -> p t c", p=128)
            wv_r = wvT.rearrange("(t p) c -> p t c", p=128)
            nc.sync.dma_start(wq_sb[:], wq_r[:])
            for hf in range(2):
                nc.sync.dma_start(xq_sb[:, hf * 4:(hf + 1) * 4, :],
                                  xq_r[:, hf * 4:(hf + 1) * 4, :])
                nc.scalar.dma_start(xk_sb[:, hf * 4:(hf + 1) * 4, :],
                                    xk_r[:, hf * 4:(hf + 1) * 4, :])
            nc.sync.dma_start(wk_sb[:], wk_r[:])
            nc.sync.dma_start(wv_sb[:], wv_r[:])
            for hf in range(2):
                nc.scalar.dma_start(xv_sb[:, hf * 4:(hf + 1) * 4, :],
                                    xv_r[:, hf * 4:(hf + 1) * 4, :])
            for pr in range(PAIRS):
                nc.sync.dma_start(wo_sb[:, pr, :],
                                  woT[pr * 128:(pr + 1) * 128, :])
            nc.sync.dma_start(vb_sb[:], bv_s[:].to_broadcast((128, CS)))
            nc.sync.dma_start(
                vh[:, :, :, HD:HD + 1],
                ones_c[:].to_broadcast((128, ST * HPC)))

            # ---- Q/K projections (transposed outputs, head-pair layout).
            # PSUM accumulates in f32; evacuation adds the bias on DVE
            # (per-partition scalar) and casts to bf16, keeping ScalarE free.
            for (x_sb, w_sb, out_sb, bias_sb) in (
                (xq_sb, wq_sb, qhT, bq_sb),
                (xk_sb, wk_sb, khT, bk_sb),
            ) if "qk" in phases else ():
                with tc.tile_pool(name="qkpsum", bufs=2, space="PSUM") as qkp:
                    ps = [qkp.tile([128, S], f32, tag="qk", name=f"qkps{i}")
                          for i in range(PAIRS)]
                    if "qkmm" in phases:
                        for kt in range(KT_PROJ):
                            for pr in range(PAIRS):
                                for qc in range(S // 512):
                                    nc.tensor.matmul(
                                        ps[pr][:, qc * 512:(qc + 1) * 512],
                                        w_sb[:, kt, pr * 128:(pr + 1) * 128],
                                        x_sb[:, kt, qc * 512:(qc + 1) * 512],
                                        start=(kt == 0),
                                        stop=(kt == KT_PROJ - 1),
                                    )
                    if "qkev" in phases:
                        for pr in range(PAIRS):
                            nc.vector.tensor_scalar_add(
                                out_sb[:, pr, :], ps[pr][:],
                                bias_sb[:, pr:pr + 1])

            # ---- attention. PSUM budget: sc 2 banks x2 + at 1 bank x2 +
            # vp/yp/rb 1 bank x2 = 8 banks.
            vap = tc.alloc_tile_pool(name="vattps", bufs=2, space="PSUM")
            dnrpool = tc.alloc_tile_pool(name="dnr", bufs=2, space="DRAM")

            def vproj(st):
                """V projection for one s-tile (natural [s, d] layout + bias),
                woven into the first attention pass so attention starts right
                after the Q/K projections."""
                vp = vap.tile([128, CS], f32, tag="vp", name="vp")
                for kt in range(KT_PROJ):
                    nc.tensor.matmul(
                        vp[:], xv_sb[:, kt, st * 128:(st + 1) * 128],
                        wv_sb[:, kt, :],
                        start=(kt == 0), stop=(kt == KT_PROJ - 1))
                nc.vector.tensor_add(
                    vh[:, st, :, 0:HD],
                    vp[:].rearrange("p (h c) -> p h c", c=HD),
                    vb_sb[:].rearrange("p (h c) -> p h c", c=HD))

            if "v" in phases and "att" not in phases:
                for st in range(ST):
                    vproj(st)

            for qck in range(S // 512) if "att" in phases else ():
                for pr in range(PAIRS):
                    q0 = qck * 512
                    weave = qck == 0 and pr == 0 and "v" in phases
                    at = [vap.tile([HD + 1, 512], f32, tag="at",
                                   name=f"at{i}") for i in range(2)]

                    def scores(kt):
                        # both heads of the pair, row-packed in PE
                        sct = vap.tile([128, 1024], f32, tag="sc", name="sct")
                        for h in range(2):
                            nc.tensor.matmul(
                                sct[:, h * 512:(h + 1) * 512],
                                khT[h * 64:(h + 1) * 64, pr,
                                    kt * 128:(kt + 1) * 128],
                                qhT[h * 64:(h + 1) * 64, pr, q0:q0 + 512],
                                start=True, stop=True)
                        return sct

                    if weave:
                        vproj(0)
                    sc_cur = scores(0)
                    for kt in range(KT_ATT):
                        ex = expool.tile([128, 1024], bf16, tag="ex")
                        nc.scalar.activation(ex[:], sc_cur[:], ACT.Exp,
                                             scale=SCALE_F8)
                        if kt + 1 < KT_ATT:
                            sc_cur = scores(kt + 1)
                            if weave:
                                vproj(kt + 1)
                        if "noav" not in phases:
                            for h in range(2):
                                nc.tensor.matmul(
                                    at[h][:],
                                    vh[:, kt, pr * 2 + h, :],
                                    ex[:, h * 512:(h + 1) * 512],
                                    start=(kt == 0), stop=(kt == KT_ATT - 1),
                                    skip_group_check=True)

                    if "evac" not in phases:
                        continue
                    # Evacuate both heads: DVE-copy the [65, 512] PSUM tiles
                    # to SBUF staging (releases the banks fast). Denominator
                    # reciprocals: the raw rows are [1, 512] (one DVE lane),
                    # where `reciprocal` is serial (~2.8us/row); bounce them
                    # through DRAM into an [8, 128] layout so the divide runs
                    # on 8 lanes (~1.1us for all four rows). The reciprocal
                    # rows are then partition-broadcast on the PE (ones [1,64]
                    # stationary x recip [1,128] moving -> [64,128] PSUM) and
                    # the normalize multiplies read them straight from PSUM.
                    # Head 1's result is partition-shifted into rows 64..127
                    # of out_sT by an SBUF->SBUF DMA.
                    stg = [spool.tile([HD + 1, 512], f32, tag="stg",
                                      name=f"stg{i}", bufs=4)
                           for i in range(2)]
                    dnr_raw = dnrpool.tile([1, 1024], f32, tag="dnr_raw")
                    for h in range(2):
                        nc.vector.tensor_copy(stg[h][:], at[h][:])
                        nc.sync.dma_start(dnr_raw[0:1, h * 512:(h + 1) * 512],
                                          stg[h][HD:HD + 1, :])
                    den8 = spool.tile([8, 128], f32, tag="den8")
                    nc.sync.dma_start(
                        den8[:],
                        dnr_raw[:].rearrange("a (p c) -> p (a c)", p=8))
                    den8r = spool.tile([8, 128], f16, tag="den8r")
                    with nc.allow_low_precision(
                            reason="f16 reciprocal feeds the PE broadcast; "
                                   "10 mantissa bits vs bf16 outputs"):
                        nc.vector.reciprocal(den8r[:], den8[:])
                    # flatten [8,128] back to one row (partition-major order
                    # == linear order) so the PE moving operand sits at
                    # base partition 0
                    rrow = spool.tile([1, 1024], f16, tag="rrow")
                    nc.sync.dma_start(rrow[:], den8r[:])
                    rb = [vap.tile([64, 512], f32, tag="vp", name=f"rb{i}")
                          for i in range(2)]
                    for h in range(2):
                        nc.tensor.matmul(
                            rb[h][:], ones_h[:],
                            rrow[0:1, h * 512:(h + 1) * 512],
                            start=True, stop=True)
                    nc.vector.tensor_mul(
                        out_sT[0:64, pr, q0:q0 + 512],
                        stg[0][0:HD, :], rb[0][:])
                    tmp = spool.tile([64, 512], bf16, tag="tmp")
                    nc.vector.tensor_mul(tmp[:], stg[1][0:HD, :], rb[1][:])
                    nc.sync.dma_start(
                        out_sT[64:128, pr, q0:q0 + 512], tmp[:])

                # ---- output projection for the four s-tiles whose out_sT
                # columns this q chunk just completed; one 1MB y write per
                # q chunk ----
                if "fin" not in phases:
                    continue
                for sti in range(4):
                    st = 4 * qck + sti
                    ysb = ypool.tile([128, DIM], bf16, tag="ysb", bufs=4)
                    for n2 in range(DIM // 512):
                        yp = vap.tile([128, 512], f32, tag="vp", name="yp")
                        for pr in range(PAIRS):
                            nc.tensor.matmul(
                                yp[:],
                                out_sT[:, pr, st * 128:(st + 1) * 128],
                                wo_sb[:, pr, n2 * 512:(n2 + 1) * 512],
                                start=(pr == 0), stop=(pr == PAIRS - 1))
                        nc.vector.tensor_copy(
                            ysb[:, n2 * 512:(n2 + 1) * 512], yp[:])
                    nc.sync.dma_start(y[st * 128:(st + 1) * 128, :], ysb[:])

            vap.release()
            dnrpool.release()

    nc.finalize()
    if split:
        _split_waits(nc, nop_templates)
    return nc


def _get_program():
    global _PROGRAM
    if _PROGRAM is None:
        _PROGRAM = _build_program()
    return _PROGRAM


def _make_in_maps(q, k, v, Wq, bq, Wk, bk, Wv, bv, Wo, bo):
    from concourse import mybir

    bf16 = mybir.dt.np(mybir.dt.bfloat16)
    f8 = mybir.dt.np(mybir.dt.float8e4)

    q = np.asarray(q, dtype=np.float32)
    k = np.asarray(k, dtype=np.float32)
    v = np.asarray(v, dtype=np.float32)
    Wq = np.asarray(Wq, dtype=np.float32)
    Wk = np.asarray(Wk, dtype=np.float32)
    Wv = np.asarray(Wv, dtype=np.float32)
    Wo = np.asarray(Wo, dtype=np.float32)
    bq = np.asarray(bq, dtype=np.float32)
    bk = np.asarray(bk, dtype=np.float32)
    bv = np.asarray(bv, dtype=np.float32)

    xT = {b: {
        "q": np.ascontiguousarray((q[b].T * 32.0).astype(f8)),
        "k": np.ascontiguousarray((k[b].T * 32.0).astype(f8)),
        "v": np.ascontiguousarray(v[b].T.astype(bf16)),
    } for b in range(B)}

    in_maps = []
    for c in range(NCORES):
        b = c // GROUPS
        g = c % GROUPS
        hs = g * CS
        in_maps.append({
            "xqT": xT[b]["q"],
            "xkT": xT[b]["k"],
            "xvT": xT[b]["v"],
            "wqT": np.ascontiguousarray((Wq[hs:hs + CS, :].T * 32.0).astype(f8)),
            "wkT": np.ascontiguousarray((Wk[hs:hs + CS, :].T * 32.0).astype(f8)),
            "wvT": np.ascontiguousarray(Wv[hs:hs + CS, :].T.astype(bf16)),
            "woT": np.ascontiguousarray(Wo[:, hs:hs + CS].T.astype(bf16)),
            "bq_s": np.ascontiguousarray(bq[hs:hs + CS].reshape(CS, 1) * 1024.0),
            "bk_s": np.ascontiguousarray(bk[hs:hs + CS].reshape(CS, 1) * 1024.0),
            "bv_s": np.ascontiguousarray(bv[hs:hs + CS].reshape(1, CS)),
            "ones_c": np.ones((1, 16 * 4), bf16),
        })
    return in_maps


def _combine(results, bo):
    bo = np.asarray(bo, dtype=np.float32)
    out = np.zeros((B, S, DIM), np.float32)
    for c in range(NCORES):
        out[c // GROUPS] += np.asarray(results[c]["y"], dtype=np.float32)
    out += bo
    return out


def run_on_hw(inputs, trace=False, **kwargs):
    """Run the kernel on the 8 NeuronCores. Returns (output, BassKernelResults)."""
    from concourse.bass_utils import run_bass_kernel_spmd

    nc = _get_program()
    in_maps = _make_in_maps(**inputs)
    res = run_bass_kernel_spmd(nc, in_maps, list(range(NCORES)),
                               trace=trace, **kwargs)
    return _combine(res.results, inputs["bo"]), res


def kernel(**inputs) -> np.ndarray:
    out, _ = run_on_hw(inputs, trace=False)
    return out
